# revision 1
# baseline (speedup 1.0000x reference)
"""Dice metric kernel for Trainium2 (Bass/Tile), 8-core data parallel.

Reference computation (per sample b):
    pred = argmax_c logits[b, :, h, w]   (softmax is monotonic -> argmax of logits)
    For classes c = 1..7:
        tps_c  = #{pred == c  and  tgt == c}
        pmc_c  = #{pred == c}
        tmc_c  = #{tgt == c}
        dice_c = 2*tps_c / (pmc_c + tmc_c + 1e-5)
    out[b] = mean_c dice_c

Device mapping (per core: 2 samples, fp16 planes [128, 2048]):
  - DVE: batched max tree (3 ops), batched is_ge -> pm masks (1 op),
    7 fused (t==c)*pm ops with free-dim accumulation (tps).
  - ACT: target histogram via Relu moments: R_k = sum relu(t-k), k=0..6;
    tgt_count_c = R_{c-1} - 2 R_c + R_{c+1}  (exact for integer t).
  - PE:  pred counts via ones-matmul over pm chunks; cross-partition sums.

Sharding: batch 16 -> 2 samples per core on 8 cores; host concatenates.
"""

import numpy as np

import concourse.bacc as bacc
import concourse.mybir as mybir
import concourse.tile as tile
from concourse.bass_utils import run_bass_kernel_spmd

B, C, H, W = 16, 8, 512, 512
NCORES = 8
BPC = B // NCORES          # samples per core
P = 128                    # SBUF partitions
F = (H * W) // P           # free dim per plane (2048)
EPS = 1e-5

_f32 = mybir.dt.float32
_f16 = mybir.dt.float16
_alu = mybir.AluOpType
_act = mybir.ActivationFunctionType


def _build_nc():
    nc = bacc.Bacc(None, target_bir_lowering=False, debug=False)
    x_dram = nc.dram_tensor("x", [BPC, C, P, F], _f16, kind="ExternalInput")
    t_dram = nc.dram_tensor("t", [BPC, P, F], _f16, kind="ExternalInput")
    o_dram = nc.dram_tensor("o", [1, BPC], _f32, kind="ExternalOutput")
    i7_dram = nc.dram_tensor("i7", [8, 8], _f32, kind="ExternalInput")

    with tile.TileContext(nc) as tc:
        with (
            tc.tile_pool(name="xp", bufs=2) as xp,
            tc.tile_pool(name="mt", bufs=1) as mtp,
            tc.tile_pool(name="wk", bufs=2) as wk,
            tc.tile_pool(name="ac", bufs=2) as acp,
            tc.tile_pool(name="cst", bufs=1) as cst,
            tc.tile_pool(name="ps", bufs=2, space="PSUM") as ps,
        ):
            ones16 = cst.tile([P, 1], _f16)
            nc.gpsimd.memset(ones16[:], 1.0)
            ones32 = cst.tile([P, 1], _f32)
            nc.gpsimd.memset(ones32[:], 1.0)
            kbias = cst.tile([P, 8], _f32)
            for k in range(7):
                nc.gpsimd.memset(kbias[:, k : k + 1], -float(k))
            ecs = cst.tile([P, 7, 8], _f16)
            nc.gpsimd.memset(ecs[:], 0.0)
            for ci in range(7):
                nc.gpsimd.memset(ecs[:, ci, ci : ci + 1], 1.0)
            osb = cst.tile([1, BPC], _f32)

            NHMAX = 2
            i7 = cst.tile([8, 8], _f32)
            for b in range(BPC):
                BOUNDS = [0, 640, F] if b == 0 else [0, F]
                NH = len(BOUNDS) - 1
                BIG = [
                    h
                    for h in range(NH)
                    if BOUNDS[h + 1] - BOUNDS[h] >= 512
                ]
                xbig = xp.tile([P, C, F], _f16, tag="x")
                tt = xp.tile([P, F], _f16, tag="t")
                # acc cols: R-block 0..17 (chunk h: 9h+k, k=0..6; 7,8,16,17
                # stay zero), tps-block 18..31 (chunk h: 18+7h+ci)
                acc = acp.tile([P, 32], _f32, tag="acc")
                nc.gpsimd.memset(acc[:], 0.0)
                pp = ps.tile([8, 512], _f32, tag="pp")
                pp0 = None

                xr = x_dram[b].rearrange("c p f -> p c f")
                for h in range(NH):
                    hs = slice(BOUNDS[h], BOUNDS[h + 1])
                    Fh = BOUNDS[h + 1] - BOUNDS[h]
                    l1 = mtp.tile([P, 4, Fh], _f16, tag="l1")
                    if b == 0 and h == 0:
                        # split the exposed first load by class pairs so the
                        # first max op starts after half the transfer
                        nc.sync.dma_start(xbig[:, 0:2, hs], xr[:, 0:2, hs])
                        nc.sync.dma_start(xbig[:, 4:6, hs], xr[:, 4:6, hs])
                        nc.sync.dma_start(xbig[:, 2:4, hs], xr[:, 2:4, hs])
                        nc.sync.dma_start(xbig[:, 6:8, hs], xr[:, 6:8, hs])
                        nc.sync.dma_start(tt[:, hs], t_dram[b, :, hs])
                        # tiny const load off the critical path
                        nc.sync.dma_start(i7[:], i7_dram[:])
                        nc.vector.tensor_max(
                            l1[:, 0:2, :], xbig[:, 0:2, hs], xbig[:, 4:6, hs]
                        )
                        nc.vector.tensor_max(
                            l1[:, 2:4, :], xbig[:, 2:4, hs], xbig[:, 6:8, hs]
                        )
                    else:
                        nc.sync.dma_start(xbig[:, :, hs], xr[:, :, hs])
                        nc.sync.dma_start(tt[:, hs], t_dram[b, :, hs])
                        nc.vector.tensor_max(
                            l1[:], xbig[:, 0:4, hs], xbig[:, 4:8, hs]
                        )
                    l2 = mtp.tile([P, 2, Fh], _f16, tag="l2")
                    nc.vector.tensor_max(l2[:], l1[:, 0:2, :], l1[:, 2:4, :])
                    mx = wk.tile([P, Fh], _f16, tag="mx")
                    nc.vector.tensor_max(mx[:], l2[:, 0, :], l2[:, 1, :])

                    # pm masks for classes 1..7 in one batched op
                    pm = mtp.tile([P, 7, Fh], _f16, tag="pm")
                    mxb = mx.rearrange("p (o f) -> p o f", o=1).broadcast_to(
                        (P, 7, Fh)
                    )
                    nc.vector.tensor_tensor(
                        out=pm[:], in0=xbig[:, 1:8, hs], in1=mxb, op=_alu.is_ge
                    )

                    for k in range(7):
                        aj = wk.tile([P, Fh], _f16, tag="aj")
                        nc.scalar.activation(
                            aj[:],
                            tt[:, hs],
                            _act.Relu,
                            bias=kbias[:, k : k + 1],
                            scale=1.0,
                            accum_out=acc[:, 9 * h + k : 9 * h + k + 1],
                        )

                    for ci in range(7):
                        junk = wk.tile([P, Fh], _f16, tag="junk")
                        nc.vector.scalar_tensor_tensor(
                            out=junk[:],
                            in0=tt[:, hs],
                            scalar=float(ci + 1),
                            in1=pm[:, ci, :],
                            op0=_alu.is_equal,
                            op1=_alu.mult,
                            accum_out=acc[:, 18 + 7 * h + ci : 19 + 7 * h + ci],
                        )

                    # pred counts: pp[ci, :] += ecs[ci]^T @ pm[:, ci, chunk]
                    if Fh < 512:
                        # small lead-in chunk: its own PSUM group
                        pp0 = ps.tile([8, 512], _f32, tag="pp0")
                        for ci in range(7):
                            nc.tensor.matmul(
                                pp0[:, 0:Fh],
                                ecs[:, ci, :],
                                pm[:, ci, :],
                                start=(ci == 0),
                                stop=(ci == 6),
                            )
                    else:
                        for ci in range(7):
                            starts = list(range(0, Fh, 512))
                            for j, js in enumerate(starts):
                                n = min(512, Fh - js)
                                nc.tensor.matmul(
                                    pp[:, 0:n],
                                    ecs[:, ci, :],
                                    pm[:, ci, js : js + n],
                                    start=(h == BIG[0] and ci == 0 and j == 0),
                                    stop=(
                                        h == BIG[-1]
                                        and ci == 6
                                        and j == len(starts) - 1
                                    ),
                                )

                pr = acp.tile([8, 1], _f32, tag="pr")
                aj2 = wk.tile([8, 512], _f32, tag="aj2")
                nc.scalar.activation(
                    aj2[:], pp[:, :], _act.Copy, accum_out=pr[:, :]
                )
                if pp0 is not None:
                    pr0 = acp.tile([8, 1], _f32, tag="pr0")
                    aj3 = wk.tile([8, 512], _f32, tag="aj3")
                    nc.scalar.activation(
                        aj3[0:8, 0:256], pp0[:, 0:256], _act.Copy, accum_out=pr0[:, :]
                    )
                    nc.vector.tensor_add(pr[:, :], pr[:, :], pr0[:, :])

                # cross-partition sums: R-block early (independent of tps)
                ptr_ = ps.tile([1, 18], _f32, tag="ptr", bufs=2)
                nc.tensor.matmul(
                    ptr_[:], ones32[:], acc[:, 0:18], start=True, stop=True
                )
                ptt = ps.tile([1, 14], _f32, tag="ptt", bufs=2)
                nc.tensor.matmul(
                    ptt[:], ones32[:], acc[:, 18:32], start=True, stop=True
                )
                pt2 = ps.tile([1, 8], _f32, tag="pt2", bufs=1)
                nc.tensor.matmul(
                    pt2[0:1, 0:7], pr[0:7, :], i7[0:7, 0:7], start=True, stop=True
                )

                # early epilogue (runs while tps stt ops still stream)
                cntr = wk.tile([1, 18], _f32, tag="cntr")
                nc.scalar.copy(cntr[:], ptr_[:])
                mr = wk.tile([1, 9], _f32, tag="mr")
                nc.vector.tensor_add(mr[:], cntr[0:1, 0:9], cntr[0:1, 9:18])
                cnt2 = wk.tile([1, 8], _f32, tag="cnt2")
                nc.scalar.copy(cnt2[0:1, 0:7], pt2[0:1, 0:7])
                v = wk.tile([1, 8], _f32, tag="v")
                nc.vector.tensor_sub(v[:], mr[0:1, 0:8], mr[0:1, 1:9])
                tmv = wk.tile([1, 8], _f32, tag="tmv")
                nc.vector.tensor_sub(tmv[0:1, 0:7], v[0:1, 0:7], v[0:1, 1:8])
                den = wk.tile([1, 8], _f32, tag="den")
                nc.vector.scalar_tensor_tensor(
                    out=den[0:1, 0:7],
                    in0=cnt2[0:1, 0:7],
                    scalar=EPS,
                    in1=tmv[0:1, 0:7],
                    op0=_alu.add,
                    op1=_alu.add,
                )
                rec = wk.tile([1, 8], _f32, tag="rec")
                nc.vector.reciprocal(rec[0:1, 0:7], den[0:1, 0:7])
                # late tail: only the tps-dependent hops
                cntt = wk.tile([1, 14], _f32, tag="cntt")
                nc.scalar.copy(cntt[:], ptt[:])
                tsum = wk.tile([1, 8], _f32, tag="tsum")
                nc.vector.tensor_add(
                    tsum[0:1, 0:7], cntt[0:1, 0:7], cntt[0:1, 7:14]
                )
                dice = wk.tile([1, 8], _f32, tag="dice")
                nc.vector.scalar_tensor_tensor(
                    out=dice[0:1, 0:7],
                    in0=tsum[0:1, 0:7],
                    scalar=2.0 / 7.0,
                    in1=rec[0:1, 0:7],
                    op0=_alu.mult,
                    op1=_alu.mult,
                    accum_out=osb[0:1, b : b + 1],
                )

            nc.sync.dma_start(o_dram[:], osb[:])

    nc.compile()
    return nc


_NC_CACHE = {}


def _get_nc():
    if "nc" not in _NC_CACHE:
        _NC_CACHE["nc"] = _build_nc()
    return _NC_CACHE["nc"]


def make_in_maps(inputs: np.ndarray, targets: np.ndarray) -> list:
    x = (
        np.ascontiguousarray(inputs, dtype=np.float32)
        .astype(np.float16)
        .reshape(NCORES, BPC, C, P, F)
    )
    t = (
        np.ascontiguousarray(targets)
        .astype(np.float16)
        .reshape(NCORES, BPC, P, F)
    )
    eye = np.eye(8, dtype=np.float32)
    return [{"x": x[i], "t": t[i], "i7": eye} for i in range(NCORES)]


def kernel(inputs: np.ndarray, targets: np.ndarray) -> np.ndarray:
    in_maps = make_in_maps(inputs, targets)
    nc = _get_nc()
    res = run_bass_kernel_spmd(nc, in_maps, list(range(NCORES)))
    outs = [res.results[i]["o"].reshape(BPC) for i in range(NCORES)]
    return np.concatenate(outs).astype(np.float32)

